# revision 3
# baseline (speedup 1.0000x reference)
"""Trainium2 Bass kernel for ContinuousIntegratedKoopmanOperator.

reference: odeint(dz/dt = z @ W) sampled at t = DT*[1..T], y0 = x at t[0].
Closed form (time-invariant linear ODE): out[:, j, :] = x @ expm(DT*j*W).

Strategy (v3 — DMA-bound, so minimize HBM bytes and ring stalls):
  host: compute Mj = expm(DT*j*W) for j=0..T-1 in float64; cast the
        (D, T*D) power table and x to fp16 (tolerance is 2e-2; fp16
        end-to-end measures ~3.6e-4 rel err).
  device (8 cores, batch-sharded 1024 rows each):
        out_tile = x @ M_block via ONE full-rate fp16 matmul per
        512-wide block (PSUM f32 accumulate over K=128).
        8 batch tiles x 16 j-blocks; PSUM rotated as 4 x 2-bank pairs;
        drains split across Vector AND Scalar engines (PSUM read port
        limits each to ~1.1us/pair), casting PSUM f32 -> fp16 staging.
        Outputs are fp16 (HALF the f32 write traffic) and upcast on host.
  rings: TRN2 has two HWDGE rings (sync + scalar), FIFO per ring. All
        input loads ride the SCALAR ring (x tile-0 slice first so PE
        starts ~1.5us in); the SYNC ring carries ONLY output stores at
        per-pair (256KB, 2KB/row) granularity so the write stream is
        never head-of-line blocked by loads or coarse drain waits.
  sync: raw bass, explicit sems; one load-sem per input DMA and
        per-engine drain sems so every wait proves a SPECIFIC event.
"""
import numpy as np

DT = 0.01
B, D, T = 8192, 128, 64
NCORES = 8
BSH = B // NCORES          # 1024 rows per core
NTILES = BSH // 128        # 8 batch tiles per core
BW = 512                   # j-block width (one PSUM bank of f32)
NBLK = (T * D) // BW       # 16 blocks per tile
NPAIR = 8                  # block-pairs per tile (drain unit = 2 banks)
NCHUNK = 8                 # M load chunks (2 blocks = 1024 cols each)
NSTG = 4                   # staging buffers (fp16: 16KB/partition each)

_CACHE = {}


def _expm_table(W: np.ndarray) -> np.ndarray:
    """(D, T*D) float64: columns [j*D:(j+1)*D] = expm(DT*j*W)."""
    A = DT * W.astype(np.float64)
    M1 = np.eye(D, dtype=np.float64)
    term = np.eye(D, dtype=np.float64)
    for n in range(1, 24):
        term = term @ A / n
        M1 += term
    Ms = np.empty((T, D, D), dtype=np.float64)
    Ms[0] = np.eye(D)
    for j in range(1, T):
        Ms[j] = Ms[j - 1] @ M1
    return np.ascontiguousarray(Ms.transpose(1, 0, 2).reshape(D, T * D))


def _build_nc():
    import concourse.bass as bass
    import concourse.mybir as mybir

    f16 = mybir.dt.float16

    nc = bass.Bass(trn_type="TRN2")
    xT_d = nc.dram_tensor("xT", (D, BSH), f16, kind="ExternalInput")
    M_d = nc.dram_tensor("M", (D, T * D), f16, kind="ExternalInput")
    out_d = nc.dram_tensor("out", (BSH, T * D), f16, kind="ExternalOutput")

    xT_s = nc.alloc_sbuf_tensor("xT_s", [D, BSH], f16)
    M_s = nc.alloc_sbuf_tensor("M_s", [D, T * D], f16)
    stg = [nc.alloc_sbuf_tensor(f"stg{p}", [128, NBLK * BW], f16) for p in range(NSTG)]
    psum = nc.alloc_psum_tensor("acc", [128, 8 * 512], mybir.dt.float32)

    s_ld = [nc.alloc_semaphore(f"s_ld{c}") for c in range(NCHUNK)]
    s_ldx0 = nc.alloc_semaphore("s_ldx0")  # x tile 0
    s_ldxr = nc.alloc_semaphore("s_ldxr")  # x tiles 1..7
    s_mm = nc.alloc_semaphore("s_mm")
    s_dv = nc.alloc_semaphore("s_dv")      # Vector drains
    s_da = nc.alloc_semaphore("s_da")      # Scalar drains
    s_osy = [nc.alloc_semaphore(f"s_osy{p}") for p in range(NSTG)]
    s_boot = nc.alloc_semaphore("s_boot")

    all_sems = [*s_ld, s_ldx0, s_ldxr, s_mm, s_dv, s_da, *s_osy, s_boot]
    nums = sorted(s.num for s in all_sems)
    assert nums == list(range(nums[0], nums[-1] + 1)), "sems not contiguous"
    sem_range = range(nums[0], nums[-1] + 1)

    nc.gpsimd.dma_reset(sem_range)

    # drain engine for pair q: even -> Vector, odd -> Scalar
    def dr_sem(q):
        return s_dv if q % 2 == 0 else s_da

    def dr_val(i, q):
        return 4 * i + q // 2 + 1  # per-engine drain count after pair (i, q)

    PW = 2 * BW  # pair width in fp16 cols (1024) = out-DMA unit

    # number of tiles with index < n mapping to staging p
    def ntile_p(p, n=NTILES):
        return len([i for i in range(n) if i % NSTG == p])

    CW = 2 * BW  # M chunk width (1024 cols)

    with nc.Block() as block:
        @block.sync
        def _(sync):
            sync.sem_clear(sem_range)
            sync.nop().then_inc(s_boot, 1)
            # per-pair outs; ring carries ONLY stores
            for i in range(NTILES):
                p = i % NSTG
                for q in range(NPAIR):
                    sync.wait_ge(dr_sem(q), dr_val(i, q))
                    sync.dma_start(
                        out=out_d[i * 128:(i + 1) * 128, q * PW:(q + 1) * PW],
                        in_=stg[p][:, q * PW:(q + 1) * PW],
                    ).then_inc(s_osy[p], 16)
            for p in range(NSTG):
                sync.wait_ge(s_osy[p], 128 * ntile_p(p))

        @block.tensor
        def _(tensor):
            tensor.wait_ge(s_boot, 1)
            tensor.wait_ge(s_ldx0, 16)
            for i in range(NTILES):
                if i == 1:
                    tensor.wait_ge(s_ldxr, 16)
                for b in range(NBLK):
                    q = b // 2                      # pair in tile
                    P = i * NPAIR + q               # global pair
                    if i == 0:
                        tensor.wait_ge(s_ld[b // 2], 16)
                    if b % 2 == 0 and P >= 4:       # pair slot reused: drain done?
                        i_, q_ = divmod(P - 4, NPAIR)
                        tensor.wait_ge(dr_sem(q_), dr_val(i_, q_))
                    pb = (P % 4) * 1024 + (b % 2) * 512
                    tensor.matmul(psum[:, pb:pb + 512],
                                  xT_s[:, i * 128:(i + 1) * 128],
                                  M_s[:, b * BW:(b + 1) * BW],
                                  start=True, stop=True).then_inc(s_mm, 1)

        def drain_stream(eng, parity):
            eng.wait_ge(s_boot, 1)
            if parity == 1:
                # scalar ring: all input loads, paced for earliest PE start
                eng.dma_start(out=xT_s[:, 0:128],
                              in_=xT_d[:, 0:128]).then_inc(s_ldx0, 16)
                eng.dma_start(out=M_s[:, 0:CW],
                              in_=M_d[:, 0:CW]).then_inc(s_ld[0], 16)
                eng.dma_start(out=xT_s[:, 128:BSH],
                              in_=xT_d[:, 128:BSH]).then_inc(s_ldxr, 16)
                for c in range(1, NCHUNK):
                    eng.dma_start(out=M_s[:, c * CW:(c + 1) * CW],
                                  in_=M_d[:, c * CW:(c + 1) * CW]
                                  ).then_inc(s_ld[c], 16)
            for i in range(NTILES):
                p = i % NSTG
                first = True
                for q in range(parity, NPAIR, 2):
                    P = i * NPAIR + q
                    if first and i >= NSTG:
                        eng.wait_ge(s_osy[p], 128 * ntile_p(p, i - NSTG + 1))
                    first = False
                    eng.wait_ge(s_mm, i * NBLK + 2 * (q + 1))  # both blocks of pair
                    pp = (P % 4) * 1024
                    sem = s_dv if parity == 0 else s_da
                    if parity == 0:
                        eng.tensor_copy(out=stg[p][:, q * 1024:(q + 1) * 1024],
                                        in_=psum[:, pp:pp + 1024]).then_inc(sem, 1)
                    else:
                        eng.copy(out=stg[p][:, q * 1024:(q + 1) * 1024],
                                 in_=psum[:, pp:pp + 1024]).then_inc(sem, 1)

        @block.vector
        def _(vector):
            drain_stream(vector, 0)

        @block.scalar
        def _(scalar):
            drain_stream(scalar, 1)

    return nc


def _prep_inputs(x: np.ndarray, Mcat64: np.ndarray):
    Mb = np.ascontiguousarray(Mcat64.astype(np.float16))
    maps = []
    for c in range(NCORES):
        xT = np.ascontiguousarray(x[c * BSH:(c + 1) * BSH].T.astype(np.float16))
        maps.append({"xT": xT, "M": Mb})
    return maps


def run_on_device(x: np.ndarray, Mcat64: np.ndarray, trace: bool = False):
    from concourse.bass_utils import run_bass_kernel_spmd

    if "nc" not in _CACHE:
        _CACHE["nc"] = _build_nc()
    nc = _CACHE["nc"]

    in_maps = _prep_inputs(x, Mcat64)
    res = run_bass_kernel_spmd(nc, in_maps, core_ids=list(range(NCORES)), trace=trace)
    out = np.empty((B, T, D), dtype=np.float32)
    for c in range(NCORES):
        out[c * BSH:(c + 1) * BSH] = (
            res.results[c]["out"].astype(np.float32).reshape(BSH, T, D))
    return out, res


def kernel(x, W, T):
    x = np.asarray(x, dtype=np.float32)
    W = np.asarray(W, dtype=np.float32)
    assert int(T) == 64 and x.shape == (B, D) and W.shape == (D, D)
    Mcat64 = _expm_table(W)
    out, _ = run_on_device(x, Mcat64, trace=False)
    return out
